# revision 4
# baseline (speedup 1.0000x reference)
"""ChebyNet (K=3, 2 layers) forward on 8 Trainium2 NeuronCores.

Strategy: node sharding. Each core owns 1280 padded rows (10000 -> 10240).
The sparse propagation  prop(t) = -D^-1/2 A D^-1/2 t  is computed as a dense
matmul against the (transposed) adjacency-count matrix AT[s, d] held
SBUF-resident in fp8e4m3 (counts are small ints -> exact). Features move in
bf16, accumulation in fp32 PSUM. The diagonal scalings dis[s]/dis[d] are
applied as per-partition scalar multiplies on the vector engine. Between
propagation hops the scaled features are AllGathered across the 8 cores.

Layer 1:  Tx0 = x; Tx1 = prop(x); Tx2 = 2*prop(Tx1) - x
          h = relu(Tx0@W1[0] + Tx1@W1[1] + Tx2@W1[2] + b1)
Layer 2:  same recursion on h with W2/b2 (no relu).
"""

import sys

for _p in ("/opt/trn_rl_repo", "/root/.axon_site", "/root/.axon_site/_ro/trn_rl_repo",
           "/root/.axon_site/_ro/pypackages"):
    if _p not in sys.path:
        sys.path.append(_p)

import numpy as np
import ml_dtypes

import concourse.bacc as bacc
import concourse.tile as tile
from concourse import bass, mybir
from concourse.bass_utils import run_bass_kernel_spmd
from concourse.masks import make_identity

# problem constants (hardcoded per harness contract)
N, E, IN, HID, OUT, K = 10000, 320000, 256, 256, 128, 3
CORES = 8
NP = 10240          # padded node count
RPC = NP // CORES   # rows per core = 1280
MB = RPC // 128     # M-blocks per core = 10
KT = NP // 128      # K-tiles = 80
F = IN              # feature width through both prop stages = 256
P = 128

FP8 = mybir.dt.float8e4
BF16 = mybir.dt.bfloat16
F32 = mybir.dt.float32

_STATE = {}


def _emit_prop(nc, psum_pool, at_tiles, u_tiles, mb, n_cols, tag):
    """Accumulate prop psum tile [128, n_cols] for M-block mb over all KT k-tiles."""
    pp = psum_pool.tile([P, n_cols], F32, tag=tag, bufs=2, name=f"{tag}_{mb}")
    sl = slice(mb * P, (mb + 1) * P)
    for kt in range(KT):
        nc.tensor.matmul(
            pp[:], at_tiles[kt][:, sl], u_tiles[kt][:, :n_cols],
            start=(kt == 0), stop=(kt == KT - 1),
        )
    return pp


def _transpose_to_bf16(nc, ps_tr, sb_tr, ident, src_ap, mb, nchunks, tag,
                       persistent=False):
    """PE-transpose src [128, nchunks*128] f32 -> list of [128,128] bf16 tiles.

    All transposes share one rotating PSUM tag; persistent results get a
    dedicated SBUF slot per (mb, chunk), transients share a rotating tag.
    """
    outs = []
    for c in range(nchunks):
        tp = ps_tr.tile([P, P], F32, tag="trps", bufs=2, name=f"{tag}ps_{mb}_{c}")
        nc.tensor.transpose(tp[:], src_ap[:, c * P:(c + 1) * P], ident[:])
        if persistent:
            tb = sb_tr.tile([P, P], BF16, tag=f"{tag}sb_{mb}_{c}", bufs=1,
                            name=f"{tag}sb_{mb}_{c}")
        else:
            tb = sb_tr.tile([P, P], BF16, tag=f"{tag}sb", bufs=4,
                            name=f"{tag}sb_{mb}_{c}")
        nc.vector.tensor_copy(tb[:], tp[:])
        outs.append(tb)
    return outs


def _build():
    nc = bacc.Bacc("TRN2", target_bir_lowering=False, debug=False, num_devices=CORES)

    # DRAM I/O (per-core data supplied via in_maps)
    at_d = nc.dram_tensor("at", [NP, RPC], FP8, kind="ExternalInput")
    u0_d = nc.dram_tensor("u0", [NP, F], BF16, kind="ExternalInput")
    xo_d = nc.dram_tensor("xo", [RPC, F], F32, kind="ExternalInput")
    xoT_d = nc.dram_tensor("xoT", [F, RPC], BF16, kind="ExternalInput")
    diso_d = nc.dram_tensor("diso", [P, MB], F32, kind="ExternalInput")
    ndiso_d = nc.dram_tensor("ndiso", [P, MB], F32, kind="ExternalInput")
    n2diso_d = nc.dram_tensor("n2diso", [P, MB], F32, kind="ExternalInput")
    w1_d = nc.dram_tensor("w1", [K, IN, HID], BF16, kind="ExternalInput")
    w2_d = nc.dram_tensor("w2", [K, HID, OUT], BF16, kind="ExternalInput")
    b1r_d = nc.dram_tensor("b1r", [P, HID], F32, kind="ExternalInput")
    b2r_d = nc.dram_tensor("b2r", [P, OUT], F32, kind="ExternalInput")
    out_d = nc.dram_tensor("outo", [RPC, OUT], F32, kind="ExternalOutput")

    at_r = at_d.ap().rearrange("(kt p) d -> kt p d", p=P)
    u0_r = u0_d.ap().rearrange("(kt p) f -> kt p f", p=P)
    xo_r = xo_d.ap().rearrange("(m p) f -> m p f", p=P)
    xoT_r = xoT_d.ap().rearrange("(c p) d -> c p d", p=P)

    with tile.TileContext(nc) as tc:
        with (
            tc.tile_pool(name="res", bufs=1) as res,        # persistent SBUF
            tc.tile_pool(name="wrk", bufs=1) as wrk,        # transient SBUF (tags set bufs)
            tc.tile_pool(name="pprop", bufs=1, space="PSUM") as pprop,
            tc.tile_pool(name="pterm", bufs=1, space="PSUM") as pterm,
            tc.tile_pool(name="ptr", bufs=1, space="PSUM") as ptr,
            tc.tile_pool(name="dram", bufs=1, space="DRAM") as dram,
        ):
            # ---- resident loads ----
            at_tiles = []
            for kt in range(KT):
                t = res.tile([P, RPC], FP8, tag=f"at{kt}", name=f"at{kt}")
                nc.sync.dma_start(t[:], at_r[kt])
                at_tiles.append(t)
            u_tiles = []
            for kt in range(KT):
                t = res.tile([P, F], BF16, tag=f"u{kt}", name=f"u{kt}")
                nc.sync.dma_start(t[:], u0_r[kt])
                u_tiles.append(t)
            xoT_t = []
            for c in range(2):
                t = res.tile([P, RPC], BF16, tag=f"xoT{c}", name=f"xoT{c}")
                nc.sync.dma_start(t[:], xoT_r[c])
                xoT_t.append(t)
            diso = res.tile([P, MB], F32, name="diso")
            nc.sync.dma_start(diso[:], diso_d[:])
            ndiso = res.tile([P, MB], F32, name="ndiso")
            nc.sync.dma_start(ndiso[:], ndiso_d[:])
            n2diso = res.tile([P, MB], F32, name="n2diso")
            nc.sync.dma_start(n2diso[:], n2diso_d[:])
            w1t = [[None, None] for _ in range(K)]
            for k in range(K):
                for c in range(2):
                    t = res.tile([P, HID], BF16, tag=f"w1_{k}_{c}", name=f"w1_{k}_{c}")
                    nc.sync.dma_start(t[:], w1_d[k, c * P:(c + 1) * P, :])
                    w1t[k][c] = t
            w2t = [[None, None] for _ in range(K)]
            for k in range(K):
                for c in range(2):
                    t = res.tile([P, OUT], BF16, tag=f"w2_{k}_{c}", name=f"w2_{k}_{c}")
                    nc.sync.dma_start(t[:], w2_d[k, c * P:(c + 1) * P, :])
                    w2t[k][c] = t
            b1r = res.tile([P, HID], F32, name="b1r")
            nc.sync.dma_start(b1r[:], b1r_d[:])
            b2r = res.tile([P, OUT], F32, name="b2r")
            nc.sync.dma_start(b2r[:], b2r_d[:])
            ident = res.tile([P, P], F32, name="ident")
            make_identity(nc, ident[:])

            # persistent per-block tensors
            h_t = [res.tile([P, F], F32, tag=f"h{m}", name=f"h{m}") for m in range(MB)]
            tx1T = [None] * MB   # [2 x (128,128) bf16] per block
            t1pT = [None] * MB

            # AG bounce buffers
            ag_in = [dram.tile([RPC, F], BF16, name=f"agin{i}") for i in range(3)]
            ag_out = [dram.tile([NP, F], BF16, name=f"agout{i}") for i in range(3)]
            ag_out_r = [a[:].rearrange("(kt p) f -> kt p f", p=P) for a in ag_out]

            def do_ag(i):
                nc.gpsimd.collective_compute(
                    "AllGather", mybir.AluOpType.bypass,
                    replica_groups=[list(range(CORES))],
                    ins=[ag_in[i][:].opt()], outs=[ag_out[i][:].opt()],
                )
                for kt in range(KT):
                    nc.sync.dma_start(u_tiles[kt][:], ag_out_r[i][kt])

            # ============ LAYER 1 ============
            # hop 1: Tx1 = -dis * (AT^T @ u);  stage dis*Tx1 for AG; keep Tx1^T bf16
            for mb in range(MB):
                pp = _emit_prop(nc, pprop, at_tiles, u_tiles, mb, F, "pp")
                tx1 = wrk.tile([P, F], F32, tag="tx1", bufs=2, name=f"tx1_{mb}")
                nc.vector.tensor_scalar_mul(tx1[:], pp[:], ndiso[:, mb:mb + 1])
                sc = wrk.tile([P, F], BF16, tag="sc", bufs=3, name=f"sc1_{mb}")
                nc.vector.tensor_scalar_mul(sc[:], tx1[:], diso[:, mb:mb + 1])
                nc.sync.dma_start(ag_in[0][mb * P:(mb + 1) * P, :], sc[:])
                tx1T[mb] = _transpose_to_bf16(nc, ptr, res, ident, tx1[:], mb, 2, "t1T", persistent=True)
            do_ag(0)

            # hop 2 + layer-1 combine
            for mb in range(MB):
                pp = _emit_prop(nc, pprop, at_tiles, u_tiles, mb, F, "pp")
                tmp = wrk.tile([P, F], F32, tag="tmp", bufs=2, name=f"tmp1_{mb}")
                nc.vector.tensor_scalar_mul(tmp[:], pp[:], n2diso[:, mb:mb + 1])
                xo = wrk.tile([P, F], F32, tag="xo", bufs=2, name=f"xo_{mb}")
                nc.sync.dma_start(xo[:], xo_r[mb])
                tx2 = wrk.tile([P, F], F32, tag="tx2", bufs=2, name=f"tx2_{mb}")
                nc.vector.tensor_sub(tx2[:], tmp[:], xo[:])
                tx2T = _transpose_to_bf16(nc, ptr, wrk, ident, tx2[:], mb, 2, "t2T")

                tp = pterm.tile([P, HID], F32, tag="tp", bufs=2, name=f"tp1_{mb}")
                sl = slice(mb * P, (mb + 1) * P)
                nc.tensor.matmul(tp[:], xoT_t[0][:, sl], w1t[0][0][:], start=True, stop=False)
                nc.tensor.matmul(tp[:], xoT_t[1][:, sl], w1t[0][1][:], start=False, stop=False)
                nc.tensor.matmul(tp[:], tx1T[mb][0][:], w1t[1][0][:], start=False, stop=False)
                nc.tensor.matmul(tp[:], tx1T[mb][1][:], w1t[1][1][:], start=False, stop=False)
                nc.tensor.matmul(tp[:], tx2T[0][:], w1t[2][0][:], start=False, stop=False)
                nc.tensor.matmul(tp[:], tx2T[1][:], w1t[2][1][:], start=False, stop=True)

                h = h_t[mb]
                nc.vector.tensor_add(h[:], tp[:], b1r[:])
                nc.vector.tensor_scalar_max(h[:], h[:], 0.0)
                sc = wrk.tile([P, F], BF16, tag="sc", bufs=3, name=f"sc2_{mb}")
                nc.vector.tensor_scalar_mul(sc[:], h[:], diso[:, mb:mb + 1])
                nc.sync.dma_start(ag_in[1][mb * P:(mb + 1) * P, :], sc[:])
            do_ag(1)

            # ============ LAYER 2 ============
            # hop 3: T1' = -dis * (AT^T @ u); stage dis*T1' ; keep T1'^T bf16
            for mb in range(MB):
                pp = _emit_prop(nc, pprop, at_tiles, u_tiles, mb, F, "pp")
                t1p = wrk.tile([P, F], F32, tag="t1p", bufs=2, name=f"t1p_{mb}")
                nc.vector.tensor_scalar_mul(t1p[:], pp[:], ndiso[:, mb:mb + 1])
                sc = wrk.tile([P, F], BF16, tag="sc", bufs=3, name=f"sc3_{mb}")
                nc.vector.tensor_scalar_mul(sc[:], t1p[:], diso[:, mb:mb + 1])
                nc.sync.dma_start(ag_in[2][mb * P:(mb + 1) * P, :], sc[:])
                t1pT[mb] = _transpose_to_bf16(nc, ptr, res, ident, t1p[:], mb, 2, "t1pT", persistent=True)
            do_ag(2)

            # hop 4 + layer-2 combine
            for mb in range(MB):
                pp = _emit_prop(nc, pprop, at_tiles, u_tiles, mb, F, "pp")
                tmp = wrk.tile([P, F], F32, tag="tmp", bufs=2, name=f"tmp2_{mb}")
                nc.vector.tensor_scalar_mul(tmp[:], pp[:], n2diso[:, mb:mb + 1])
                t2p = wrk.tile([P, F], F32, tag="t2p", bufs=2, name=f"t2p_{mb}")
                nc.vector.tensor_sub(t2p[:], tmp[:], h_t[mb][:])
                hT = _transpose_to_bf16(nc, ptr, wrk, ident, h_t[mb][:], mb, 2, "hT")
                t2pT = _transpose_to_bf16(nc, ptr, wrk, ident, t2p[:], mb, 2, "t2pT")

                tp = pterm.tile([P, OUT], F32, tag="tp", bufs=2, name=f"tp2_{mb}")
                nc.tensor.matmul(tp[:], hT[0][:], w2t[0][0][:], start=True, stop=False)
                nc.tensor.matmul(tp[:], hT[1][:], w2t[0][1][:], start=False, stop=False)
                nc.tensor.matmul(tp[:], t1pT[mb][0][:], w2t[1][0][:], start=False, stop=False)
                nc.tensor.matmul(tp[:], t1pT[mb][1][:], w2t[1][1][:], start=False, stop=False)
                nc.tensor.matmul(tp[:], t2pT[0][:], w2t[2][0][:], start=False, stop=False)
                nc.tensor.matmul(tp[:], t2pT[1][:], w2t[2][1][:], start=False, stop=True)

                oacc = wrk.tile([P, OUT], F32, tag="oacc", bufs=3, name=f"oacc_{mb}")
                nc.vector.tensor_add(oacc[:], tp[:], b2r[:])
                nc.sync.dma_start(out_d[mb * P:(mb + 1) * P, :], oacc[:])

    nc.compile()
    return nc


def _prepare_inputs(x, edge, W1, b1, W2, b2):
    x = np.asarray(x, np.float32)
    edge = np.asarray(edge)
    W1 = np.asarray(W1, np.float32)
    b1 = np.asarray(b1, np.float32)
    W2 = np.asarray(W2, np.float32)
    b2 = np.asarray(b2, np.float32)
    src = edge[0].astype(np.int64)
    dst = edge[1].astype(np.int64)

    deg = np.bincount(dst, minlength=N).astype(np.float32)
    dis = np.where(deg > 0, 1.0 / np.sqrt(np.maximum(deg, 1.0)), 0.0).astype(np.float32)

    # dense transposed adjacency counts AT[s, d]
    flat = src * NP + dst
    uniq, cnt = np.unique(flat, return_counts=True)
    at8 = np.zeros(NP * NP, dtype=ml_dtypes.float8_e4m3)
    at8[uniq] = cnt.astype(ml_dtypes.float8_e4m3)
    at8 = at8.reshape(NP, NP)

    dis_pad = np.zeros(NP, np.float32)
    dis_pad[:N] = dis
    u0 = np.zeros((NP, F), ml_dtypes.bfloat16)
    u0[:N] = (dis[:, None] * x).astype(ml_dtypes.bfloat16)

    x_pad = np.zeros((NP, F), np.float32)
    x_pad[:N] = x

    w1b = W1.astype(ml_dtypes.bfloat16)
    w2b = W2.astype(ml_dtypes.bfloat16)
    b1r = np.broadcast_to(b1, (P, HID)).copy()
    b2r = np.broadcast_to(b2, (P, OUT)).copy()

    in_maps = []
    for c in range(CORES):
        rows = slice(c * RPC, (c + 1) * RPC)
        dv = dis_pad[rows]
        m = {
            "at": np.ascontiguousarray(at8[:, rows]),
            "u0": u0,
            "xo": np.ascontiguousarray(x_pad[rows]),
            "xoT": np.ascontiguousarray(x_pad[rows].T).astype(ml_dtypes.bfloat16),
            "diso": np.ascontiguousarray(dv.reshape(MB, P).T),
            "ndiso": np.ascontiguousarray((-dv).reshape(MB, P).T),
            "n2diso": np.ascontiguousarray((-2.0 * dv).reshape(MB, P).T),
            "w1": w1b,
            "w2": w2b,
            "b1r": b1r,
            "b2r": b2r,
        }
        in_maps.append(m)
    return in_maps


def _run(in_maps, trace=False, **kw):
    if "nc" not in _STATE:
        _STATE["nc"] = _build()
    r = run_bass_kernel_spmd(_STATE["nc"], in_maps, core_ids=list(range(CORES)),
                             trace=trace, **kw)
    out = np.concatenate([r.results[c]["outo"] for c in range(CORES)], axis=0)
    return out[:N], r


def kernel(**inputs) -> np.ndarray:
    in_maps = _prepare_inputs(**inputs)
    out, _ = _run(in_maps)
    return out


# revision 9
# speedup vs baseline: 1.3296x; 1.3296x over previous
"""ChebyNet (K=3, 2 layers) forward on 8 Trainium2 NeuronCores.

Strategy: node sharding. Each core owns 1280 padded rows (10000 -> 10240).
The sparse propagation  prop(t) = -D^-1/2 A D^-1/2 t  is computed as a dense
matmul against the (transposed) adjacency-count matrix AT[s, d] held
SBUF-resident in fp8e4m3 (counts are small ints -> exact). Features move in
bf16, accumulation in fp32 PSUM. The diagonal scalings dis[s]/dis[d] are
applied as per-partition scalar multiplies on the vector engine. Between
propagation hops the scaled features are AllGathered across the 8 cores;
each AllGather is split into two half-shard collectives so the wire time
overlaps with compute on both sides of the hop boundary.

Layer 1 (direct recursion):
    Tx1 = prop(x); Tx2 = 2*prop(Tx1) - x
    h = relu(x@W1[0] + Tx1@W1[1] + Tx2@W1[2] + b1)
Layer 2 (restructured -- prop commutes with the feature matmul):
    z1 = h@W2[1]; z2 = h@(2*W2[2])            (z = [z1|z2], 256 cols)
    out = h@(W2[0]-W2[2]) + prop(z)[:,:128] + prop(prop(z)[:,128:]) + b2
which halves the streamed columns of the final hop.
"""

import sys

for _p in ("/opt/trn_rl_repo", "/root/.axon_site", "/root/.axon_site/_ro/trn_rl_repo",
           "/root/.axon_site/_ro/pypackages"):
    if _p not in sys.path:
        sys.path.append(_p)

import numpy as np
import ml_dtypes

import concourse.bacc as bacc
import concourse.tile as tile
from concourse import bass, mybir
from concourse.bass_utils import run_bass_kernel_spmd
from concourse.masks import make_identity

# problem constants (hardcoded per harness contract)
N, E, IN, HID, OUT, K = 10000, 320000, 256, 256, 128, 3
CORES = 8
NP = 10240          # padded node count
RPC = NP // CORES   # rows per core = 1280
MB = RPC // 128     # M-blocks per core = 10
MBH = MB // 2       # half of the M-blocks = 5
KT = NP // 128      # K-tiles = 80
F = IN              # feature width through prop stages = 256
P = 128

FP8 = mybir.dt.float8e4
BF16 = mybir.dt.bfloat16
F32 = mybir.dt.float32

_STATE = {}


def _kt_order(split):
    """kt sweep order: with split=True consume first-half-AG tiles (kt%10<5)
    before second-half tiles, so matmuls can start after the first half-shard
    AllGather lands."""
    if not split:
        return list(range(KT))
    return [kt for kt in range(KT) if kt % MB < MBH] + \
           [kt for kt in range(KT) if kt % MB >= MBH]


def _emit_prop(nc, psum_pool, at_tiles, u_tiles, mb, n_cols, split, bufs=3):
    """Accumulate prop psum tile [128, n_cols] for M-block mb over all KT k-tiles."""
    pp = psum_pool.tile([P, n_cols], F32, tag="pp", bufs=bufs, name=f"pp_{mb}")
    sl = slice(mb * P, (mb + 1) * P)
    order = _kt_order(split)
    for j, kt in enumerate(order):
        nc.tensor.matmul(
            pp[:], at_tiles[kt][:, sl], u_tiles[kt][:, :n_cols],
            start=(j == 0), stop=(j == KT - 1),
        )
    return pp


def _transpose_to_bf16(nc, ps_tr, sb_tr, ident, src_ap, mb, nchunks, tag,
                       persistent=False):
    """PE-transpose src [128, nchunks*128] f32 -> list of [128,128] bf16 tiles."""
    outs = []
    for c in range(nchunks):
        tp = ps_tr.tile([P, P], F32, tag="trps", bufs=2, name=f"{tag}ps_{mb}_{c}")
        nc.tensor.transpose(tp[:], src_ap[:, c * P:(c + 1) * P], ident[:])
        if persistent:
            tb = sb_tr.tile([P, P], BF16, tag=f"{tag}sb_{mb}_{c}", bufs=1,
                            name=f"{tag}sb_{mb}_{c}")
        else:
            tb = sb_tr.tile([P, P], BF16, tag=f"{tag}sb", bufs=4,
                            name=f"{tag}sb_{mb}_{c}")
        nc.vector.tensor_copy(tb[:], tp[:])
        outs.append(tb)
    return outs


def _build():
    nc = bacc.Bacc("TRN2", target_bir_lowering=False, debug=False, num_devices=CORES)

    # DRAM I/O (per-core data supplied via in_maps)
    at_d = nc.dram_tensor("at", [NP, RPC], FP8, kind="ExternalInput")
    u0_d = nc.dram_tensor("u0", [NP, F], BF16, kind="ExternalInput")
    xo_d = nc.dram_tensor("xo", [RPC, F], F32, kind="ExternalInput")
    xoT_d = nc.dram_tensor("xoT", [F, RPC], BF16, kind="ExternalInput")
    diso_d = nc.dram_tensor("diso", [P, MB], F32, kind="ExternalInput")
    ndiso_d = nc.dram_tensor("ndiso", [P, MB], F32, kind="ExternalInput")
    n2diso_d = nc.dram_tensor("n2diso", [P, MB], F32, kind="ExternalInput")
    w1_d = nc.dram_tensor("w1", [K, IN, HID], BF16, kind="ExternalInput")
    # w2x[0] = W2[0]-W2[2], w2x[1] = W2[1], w2x[2] = 2*W2[2]
    w2x_d = nc.dram_tensor("w2x", [K, HID, OUT], BF16, kind="ExternalInput")
    b1r_d = nc.dram_tensor("b1r", [P, HID], F32, kind="ExternalInput")
    b2r_d = nc.dram_tensor("b2r", [P, OUT], F32, kind="ExternalInput")
    out_d = nc.dram_tensor("outo", [RPC, OUT], F32, kind="ExternalOutput")

    at_r = at_d.ap().rearrange("(kt p) d -> kt p d", p=P)
    u0_r = u0_d.ap().rearrange("(kt p) f -> kt p f", p=P)
    xo_r = xo_d.ap().rearrange("(m p) f -> m p f", p=P)
    xoT_r = xoT_d.ap().rearrange("(c p) d -> c p d", p=P)

    with tile.TileContext(nc) as tc:
        with (
            tc.tile_pool(name="res", bufs=1) as res,        # persistent SBUF
            tc.tile_pool(name="wrk", bufs=1) as wrk,        # transient SBUF (tags set bufs)
            tc.tile_pool(name="pprop", bufs=1, space="PSUM") as pprop,
            tc.tile_pool(name="pterm", bufs=1, space="PSUM") as pterm,
            tc.tile_pool(name="ptr", bufs=1, space="PSUM") as ptr,
            tc.tile_pool(name="dram", bufs=1, space="DRAM") as dram,
        ):
            # ---- resident loads (at/u interleaved so hop 1 starts early) ----
            at_tiles = []
            u_tiles = []
            for kt in range(KT):
                t = res.tile([P, RPC], FP8, tag=f"at{kt}", name=f"at{kt}")
                nc.sync.dma_start(t[:], at_r[kt])
                at_tiles.append(t)
                u = res.tile([P, F], BF16, tag=f"u{kt}", name=f"u{kt}")
                nc.sync.dma_start(u[:], u0_r[kt])
                u_tiles.append(u)
            xoT_t = []
            for c in range(2):
                t = res.tile([P, RPC], BF16, tag=f"xoT{c}", name=f"xoT{c}")
                nc.sync.dma_start(t[:], xoT_r[c])
                xoT_t.append(t)
            diso = res.tile([P, MB], F32, name="diso")
            nc.sync.dma_start(diso[:], diso_d[:])
            ndiso = res.tile([P, MB], F32, name="ndiso")
            nc.sync.dma_start(ndiso[:], ndiso_d[:])
            n2diso = res.tile([P, MB], F32, name="n2diso")
            nc.sync.dma_start(n2diso[:], n2diso_d[:])
            w1t = [[None, None] for _ in range(K)]
            for k in range(K):
                for c in range(2):
                    t = res.tile([P, HID], BF16, tag=f"w1_{k}_{c}", name=f"w1_{k}_{c}")
                    nc.sync.dma_start(t[:], w1_d[k, c * P:(c + 1) * P, :])
                    w1t[k][c] = t
            w2t = [[None, None] for _ in range(K)]
            for k in range(K):
                for c in range(2):
                    t = res.tile([P, OUT], BF16, tag=f"w2_{k}_{c}", name=f"w2_{k}_{c}")
                    nc.sync.dma_start(t[:], w2x_d[k, c * P:(c + 1) * P, :])
                    w2t[k][c] = t
            b1r = res.tile([P, HID], F32, name="b1r")
            nc.sync.dma_start(b1r[:], b1r_d[:])
            b2r = res.tile([P, OUT], F32, name="b2r")
            nc.sync.dma_start(b2r[:], b2r_d[:])
            ident = res.tile([P, P], F32, name="ident")
            make_identity(nc, ident[:])

            # persistent per-block tensors
            h_t = [res.tile([P, F], F32, tag=f"h{m}", name=f"h{m}") for m in range(MB)]
            tx1T = [None] * MB   # [2 x (128,128) bf16] per block
            hw_all = [None] * MB

            # AG bounce buffers: [round][half]
            HR = MBH * P  # rows per half-shard = 640
            AGW = [F, F, OUT]  # payload width per AG round
            ag_in = [[dram.tile([HR, AGW[i]], BF16, name=f"agin{i}{h}")
                      for h in range(2)] for i in range(3)]
            ag_out = [[dram.tile([CORES * HR, AGW[i]], BF16, name=f"agout{i}{h}")
                       for h in range(2)] for i in range(3)]

            def stage_ag(i, mb, src_ap):
                half, m = mb // MBH, mb % MBH
                nc.sync.dma_start(ag_in[i][half][m * P:(m + 1) * P, :], src_ap)

            def emit_ag(i, half):
                nc.gpsimd.collective_compute(
                    "AllGather", mybir.AluOpType.bypass,
                    replica_groups=[list(range(CORES))],
                    ins=[ag_in[i][half][:].opt()],
                    outs=[ag_out[i][half][:].opt()],
                )

            def reload_u(i):
                # u[kt] rows are core (kt*128)//1280, m-block kt%10
                n_cols = AGW[i]
                for kt in range(KT):
                    c8, m = kt // MB, kt % MB
                    half, mh = (0, m) if m < MBH else (1, m - MBH)
                    src = ag_out[i][half][c8 * HR + mh * P: c8 * HR + (mh + 1) * P, :]
                    nc.sync.dma_start(u_tiles[kt][:, :n_cols], src)

            # ============ LAYER 1 ============
            # hop 1: Tx1 = -dis * (AT^T @ u0); stage dis*Tx1; keep Tx1^T bf16
            for mb in range(MB):
                pp = _emit_prop(nc, pprop, at_tiles, u_tiles, mb, F, split=False)
                tx1 = wrk.tile([P, F], F32, tag="tx1", bufs=2, name=f"tx1_{mb}")
                nc.vector.tensor_scalar_mul(tx1[:], pp[:], ndiso[:, mb:mb + 1])
                sc = wrk.tile([P, F], BF16, tag="sc", bufs=3, name=f"sc1_{mb}")
                nc.vector.tensor_scalar_mul(sc[:], tx1[:], diso[:, mb:mb + 1])
                stage_ag(0, mb, sc[:])
                tx1T[mb] = _transpose_to_bf16(nc, ptr, res, ident, tx1[:], mb, 2,
                                              "t1T", persistent=True)
                if mb == MBH - 1:
                    emit_ag(0, 0)
            emit_ag(0, 1)
            reload_u(0)

            # hop 2 + layer-1 combine -> h, z = [h@W2[1] | h@(2 W2[2])] staged for AG1
            for mb in range(MB):
                pp = _emit_prop(nc, pprop, at_tiles, u_tiles, mb, F, split=True)
                tmp = wrk.tile([P, F], F32, tag="tmp", bufs=2, name=f"tmp1_{mb}")
                nc.vector.tensor_scalar_mul(tmp[:], pp[:], n2diso[:, mb:mb + 1])
                xo = wrk.tile([P, F], F32, tag="xo", bufs=2, name=f"xo_{mb}")
                nc.sync.dma_start(xo[:], xo_r[mb])
                tx2 = wrk.tile([P, F], F32, tag="tx2", bufs=2, name=f"tx2_{mb}")
                nc.vector.tensor_sub(tx2[:], tmp[:], xo[:])
                tx2T = _transpose_to_bf16(nc, ptr, wrk, ident, tx2[:], mb, 2, "t2T")

                tp = pterm.tile([P, HID], F32, tag="tp", bufs=2, name=f"tp1_{mb}")
                sl = slice(mb * P, (mb + 1) * P)
                nc.tensor.matmul(tp[:], xoT_t[0][:, sl], w1t[0][0][:], start=True, stop=False)
                nc.tensor.matmul(tp[:], xoT_t[1][:, sl], w1t[0][1][:], start=False, stop=False)
                nc.tensor.matmul(tp[:], tx1T[mb][0][:], w1t[1][0][:], start=False, stop=False)
                nc.tensor.matmul(tp[:], tx1T[mb][1][:], w1t[1][1][:], start=False, stop=False)
                nc.tensor.matmul(tp[:], tx2T[0][:], w1t[2][0][:], start=False, stop=False)
                nc.tensor.matmul(tp[:], tx2T[1][:], w1t[2][1][:], start=False, stop=True)

                h = h_t[mb]
                nc.vector.tensor_add(h[:], tp[:], b1r[:])
                nc.vector.tensor_scalar_max(h[:], h[:], 0.0)
                # z = [h@W2[1] | h@(2*W2[2])]  (layer-2 pre-propagation features)
                hT = _transpose_to_bf16(nc, ptr, wrk, ident, h[:], mb, 2, "hT")
                zp = pterm.tile([P, F], F32, tag="tp", bufs=2, name=f"zp_{mb}")
                nc.tensor.matmul(zp[:, 0:OUT], hT[0][:], w2t[1][0][:], start=True, stop=False)
                nc.tensor.matmul(zp[:, 0:OUT], hT[1][:], w2t[1][1][:], start=False, stop=True)
                nc.tensor.matmul(zp[:, OUT:F], hT[0][:], w2t[2][0][:], start=True, stop=False)
                nc.tensor.matmul(zp[:, OUT:F], hT[1][:], w2t[2][1][:], start=False, stop=True)
                # hw = h @ (W2[0]-W2[2]) -> keep for final combine
                hw = res.tile([P, OUT], F32, tag=f"hw{mb}", name=f"hw_{mb}")
                hw_all[mb] = hw
                hwp = ptr.tile([P, OUT], F32, tag="trps", bufs=2, name=f"hwp_{mb}")
                nc.tensor.matmul(hwp[:], hT[0][:], w2t[0][0][:], start=True, stop=False)
                nc.tensor.matmul(hwp[:], hT[1][:], w2t[0][1][:], start=False, stop=True)
                nc.vector.tensor_copy(hw[:], hwp[:])
                sc = wrk.tile([P, F], BF16, tag="sc", bufs=3, name=f"sc2_{mb}")
                nc.vector.tensor_scalar_mul(sc[:], zp[:], diso[:, mb:mb + 1])
                stage_ag(1, mb, sc[:])
                if mb == MBH - 1:
                    emit_ag(1, 0)
            emit_ag(1, 1)
            reload_u(1)

            # ============ LAYER 2 ============
            # hop 3: Lz = -dis*(AT^T @ u); col 0:128 = Lz1 (final term),
            #        col 128:256 = Lz2 -> scale+stage for AG2
            lz1 = [res.tile([P, OUT], F32, tag=f"lz1_{m}", name=f"lz1_{m}")
                   for m in range(MB)]
            for mb in range(MB):
                pp = _emit_prop(nc, pprop, at_tiles, u_tiles, mb, F, split=True)
                lz = wrk.tile([P, F], F32, tag="lz", bufs=2, name=f"lz_{mb}")
                nc.vector.tensor_scalar_mul(lz[:], pp[:], ndiso[:, mb:mb + 1])
                nc.vector.tensor_copy(lz1[mb][:], lz[:, 0:OUT])
                sc = wrk.tile([P, OUT], BF16, tag="sc3", bufs=3, name=f"sc3_{mb}")
                nc.vector.tensor_scalar_mul(sc[:], lz[:, OUT:F], diso[:, mb:mb + 1])
                stage_ag(2, mb, sc[:])
                if mb == MBH - 1:
                    emit_ag(2, 0)
            emit_ag(2, 1)
            reload_u(2)

            # hop 4 (128 cols) + final combine
            for mb in range(MB):
                pp = _emit_prop(nc, pprop, at_tiles, u_tiles, mb, OUT, split=True)
                oacc = wrk.tile([P, OUT], F32, tag="oacc", bufs=3, name=f"oacc_{mb}")
                # oacc = -dis*pp + lz1 + hw + b2
                nc.vector.tensor_scalar_mul(oacc[:], pp[:], ndiso[:, mb:mb + 1])
                nc.vector.tensor_add(oacc[:], oacc[:], lz1[mb][:])
                nc.vector.tensor_add(oacc[:], oacc[:], hw_all[mb][:])
                nc.vector.tensor_add(oacc[:], oacc[:], b2r[:])
                nc.sync.dma_start(out_d[mb * P:(mb + 1) * P, :], oacc[:])

    nc.compile()
    return nc


def _prepare_inputs(x, edge, W1, b1, W2, b2):
    x = np.asarray(x, np.float32)
    edge = np.asarray(edge)
    W1 = np.asarray(W1, np.float32)
    b1 = np.asarray(b1, np.float32)
    W2 = np.asarray(W2, np.float32)
    b2 = np.asarray(b2, np.float32)
    src = edge[0].astype(np.int64)
    dst = edge[1].astype(np.int64)

    deg = np.bincount(dst, minlength=N).astype(np.float32)
    dis = np.where(deg > 0, 1.0 / np.sqrt(np.maximum(deg, 1.0)), 0.0).astype(np.float32)

    # dense transposed adjacency counts AT[s, d]
    flat = src * NP + dst
    uniq, cnt = np.unique(flat, return_counts=True)
    at8 = np.zeros(NP * NP, dtype=ml_dtypes.float8_e4m3)
    at8[uniq] = cnt.astype(ml_dtypes.float8_e4m3)
    at8 = at8.reshape(NP, NP)

    dis_pad = np.zeros(NP, np.float32)
    dis_pad[:N] = dis
    u0 = np.zeros((NP, F), ml_dtypes.bfloat16)
    u0[:N] = (dis[:, None] * x).astype(ml_dtypes.bfloat16)

    x_pad = np.zeros((NP, F), np.float32)
    x_pad[:N] = x

    w1b = W1.astype(ml_dtypes.bfloat16)
    w2x = np.stack([W2[0] - W2[2], W2[1], 2.0 * W2[2]]).astype(ml_dtypes.bfloat16)
    b1r = np.broadcast_to(b1, (P, HID)).copy()
    b2r = np.broadcast_to(b2, (P, OUT)).copy()

    in_maps = []
    for c in range(CORES):
        rows = slice(c * RPC, (c + 1) * RPC)
        dv = dis_pad[rows]
        m = {
            "at": np.ascontiguousarray(at8[:, rows]),
            "u0": u0,
            "xo": np.ascontiguousarray(x_pad[rows]),
            "xoT": np.ascontiguousarray(x_pad[rows].T).astype(ml_dtypes.bfloat16),
            "diso": np.ascontiguousarray(dv.reshape(MB, P).T),
            "ndiso": np.ascontiguousarray((-dv).reshape(MB, P).T),
            "n2diso": np.ascontiguousarray((-2.0 * dv).reshape(MB, P).T),
            "w1": w1b,
            "w2x": w2x,
            "b1r": b1r,
            "b2r": b2r,
        }
        in_maps.append(m)
    return in_maps


def _run(in_maps, trace=False, **kw):
    if "nc" not in _STATE:
        _STATE["nc"] = _build()
    r = run_bass_kernel_spmd(_STATE["nc"], in_maps, core_ids=list(range(CORES)),
                             trace=trace, **kw)
    out = np.concatenate([r.results[c]["outo"] for c in range(CORES)], axis=0)
    return out[:N], r


def kernel(**inputs) -> np.ndarray:
    in_maps = _prepare_inputs(**inputs)
    out, _ = _run(in_maps)
    return out


# revision 10
# speedup vs baseline: 1.4534x; 1.0932x over previous
"""ChebyNet (K=3, 2 layers) forward on 8 Trainium2 NeuronCores.

Strategy: node sharding. Each core owns 1280 padded rows (10000 -> 10240).
The sparse propagation  prop(t) = -D^-1/2 A D^-1/2 t  is computed as a dense
matmul against the (transposed) adjacency-count matrix AT[s, d] held
SBUF-resident in fp8e4m3 (counts are small ints -> exact). Features move in
bf16, accumulation in fp32 PSUM. The diagonal scalings dis[s]/dis[d] are
applied as per-partition scalar multiplies on the vector engine. Between
propagation hops the scaled features are AllGathered across the 8 cores;
each AllGather is split into two half-shard collectives so the wire time
overlaps with compute on both sides of the hop boundary.

Layer 1 (direct recursion):
    Tx1 = prop(x); Tx2 = 2*prop(Tx1) - x
    h = relu(x@W1[0] + Tx1@W1[1] + Tx2@W1[2] + b1)
Layer 2 (restructured -- prop commutes with the feature matmul):
    z1 = h@W2[1]; z2 = h@(2*W2[2])            (z = [z1|z2], 256 cols)
    out = h@(W2[0]-W2[2]) + prop(z)[:,:128] + prop(prop(z)[:,128:]) + b2
which halves the streamed columns of the final hop.
"""

import sys

for _p in ("/opt/trn_rl_repo", "/root/.axon_site", "/root/.axon_site/_ro/trn_rl_repo",
           "/root/.axon_site/_ro/pypackages"):
    if _p not in sys.path:
        sys.path.append(_p)

import numpy as np
import ml_dtypes

import concourse.bacc as bacc
import concourse.tile as tile
from concourse import bass, mybir
from concourse.bass_utils import run_bass_kernel_spmd
from concourse.masks import make_identity

# problem constants (hardcoded per harness contract)
N, E, IN, HID, OUT, K = 10000, 320000, 256, 256, 128, 3
CORES = 8
NP = 10240          # padded node count
RPC = NP // CORES   # rows per core = 1280
MB = RPC // 128     # M-blocks per core = 10
MBH = MB // 2       # half of the M-blocks = 5
KT = NP // 128      # K-tiles = 80
F = IN              # feature width through prop stages = 256
P = 128

FP8 = mybir.dt.float8e4
BF16 = mybir.dt.bfloat16
F32 = mybir.dt.float32

_STATE = {}


def _kt_order(split):
    """kt sweep order: with split=True consume first-half-AG tiles (kt%10<5)
    before second-half tiles, so matmuls can start after the first half-shard
    AllGather lands."""
    if not split:
        return list(range(KT))
    return [kt for kt in range(KT) if kt % MB < MBH] + \
           [kt for kt in range(KT) if kt % MB >= MBH]


def _emit_prop(nc, psum_pool, at_tiles, u_tiles, mb, n_cols, split, bufs=4):
    """Accumulate prop psum tile [128, n_cols] for M-block mb over all KT k-tiles."""
    pp = psum_pool.tile([P, n_cols], F32, tag="pp", bufs=bufs, name=f"pp_{mb}")
    sl = slice(mb * P, (mb + 1) * P)
    order = _kt_order(split)
    for j, kt in enumerate(order):
        nc.tensor.matmul(
            pp[:], at_tiles[kt][:, sl], u_tiles[kt][:, :n_cols],
            start=(j == 0), stop=(j == KT - 1),
        )
    return pp


def _transpose_to_bf16(nc, ps_tr, sb_tr, ident, src_ap, mb, nchunks, tag,
                       persistent=False):
    """PE-transpose src [128, nchunks*128] f32 -> list of [128,128] bf16 tiles."""
    outs = []
    for c in range(nchunks):
        tp = ps_tr.tile([P, P], F32, tag="trps", bufs=2, name=f"{tag}ps_{mb}_{c}")
        nc.tensor.transpose(tp[:], src_ap[:, c * P:(c + 1) * P], ident[:])
        if persistent:
            tb = sb_tr.tile([P, P], BF16, tag=f"{tag}sb_{mb}_{c}", bufs=1,
                            name=f"{tag}sb_{mb}_{c}")
        else:
            tb = sb_tr.tile([P, P], BF16, tag=f"{tag}sb", bufs=4,
                            name=f"{tag}sb_{mb}_{c}")
        nc.vector.tensor_copy(tb[:], tp[:])
        outs.append(tb)
    return outs


def _build():
    nc = bacc.Bacc("TRN2", target_bir_lowering=False, debug=False, num_devices=CORES)

    # DRAM I/O (per-core data supplied via in_maps)
    at_d = nc.dram_tensor("at", [NP, RPC], FP8, kind="ExternalInput")
    u0_d = nc.dram_tensor("u0", [NP, F], BF16, kind="ExternalInput")
    xo_d = nc.dram_tensor("xo", [RPC, F], F32, kind="ExternalInput")
    xoT_d = nc.dram_tensor("xoT", [F, RPC], BF16, kind="ExternalInput")
    diso_d = nc.dram_tensor("diso", [P, MB], F32, kind="ExternalInput")
    ndiso_d = nc.dram_tensor("ndiso", [P, MB], F32, kind="ExternalInput")
    n2diso_d = nc.dram_tensor("n2diso", [P, MB], F32, kind="ExternalInput")
    w1_d = nc.dram_tensor("w1", [K, IN, HID], BF16, kind="ExternalInput")
    # w2x[0] = W2[0]-W2[2], w2x[1] = W2[1], w2x[2] = 2*W2[2]
    w2x_d = nc.dram_tensor("w2x", [K, HID, OUT], BF16, kind="ExternalInput")
    b1r_d = nc.dram_tensor("b1r", [P, HID], F32, kind="ExternalInput")
    b2r_d = nc.dram_tensor("b2r", [P, OUT], F32, kind="ExternalInput")
    out_d = nc.dram_tensor("outo", [RPC, OUT], F32, kind="ExternalOutput")

    at_r = at_d.ap().rearrange("(kt p) d -> kt p d", p=P)
    u0_r = u0_d.ap().rearrange("(kt p) f -> kt p f", p=P)
    xo_r = xo_d.ap().rearrange("(m p) f -> m p f", p=P)
    xoT_r = xoT_d.ap().rearrange("(c p) d -> c p d", p=P)

    with tile.TileContext(nc) as tc:
        with (
            tc.tile_pool(name="res", bufs=1) as res,        # persistent SBUF
            tc.tile_pool(name="wrk", bufs=1) as wrk,        # transient SBUF (tags set bufs)
            tc.tile_pool(name="pprop", bufs=1, space="PSUM") as pprop,
            tc.tile_pool(name="pterm", bufs=1, space="PSUM") as pterm,
            tc.tile_pool(name="ptr", bufs=1, space="PSUM") as ptr,
            tc.tile_pool(name="dram", bufs=1, space="DRAM") as dram,
        ):
            # ---- resident loads (at/u interleaved so hop 1 starts early) ----
            at_tiles = []
            u_tiles = []
            for kt in range(KT):
                t = res.tile([P, RPC], FP8, tag=f"at{kt}", name=f"at{kt}")
                nc.sync.dma_start(t[:], at_r[kt])
                at_tiles.append(t)
                u = res.tile([P, F], BF16, tag=f"u{kt}", name=f"u{kt}")
                nc.sync.dma_start(u[:], u0_r[kt])
                u_tiles.append(u)
            xoT_t = []
            for c in range(2):
                t = res.tile([P, RPC], BF16, tag=f"xoT{c}", name=f"xoT{c}")
                nc.sync.dma_start(t[:], xoT_r[c])
                xoT_t.append(t)
            diso = res.tile([P, MB], F32, name="diso")
            nc.sync.dma_start(diso[:], diso_d[:])
            ndiso = res.tile([P, MB], F32, name="ndiso")
            nc.sync.dma_start(ndiso[:], ndiso_d[:])
            n2diso = res.tile([P, MB], F32, name="n2diso")
            nc.sync.dma_start(n2diso[:], n2diso_d[:])
            w1t = [[None, None] for _ in range(K)]
            for k in range(K):
                for c in range(2):
                    t = res.tile([P, HID], BF16, tag=f"w1_{k}_{c}", name=f"w1_{k}_{c}")
                    nc.sync.dma_start(t[:], w1_d[k, c * P:(c + 1) * P, :])
                    w1t[k][c] = t
            w2t = [[None, None] for _ in range(K)]
            for k in range(K):
                for c in range(2):
                    t = res.tile([P, OUT], BF16, tag=f"w2_{k}_{c}", name=f"w2_{k}_{c}")
                    nc.sync.dma_start(t[:], w2x_d[k, c * P:(c + 1) * P, :])
                    w2t[k][c] = t
            b1r = res.tile([P, HID], F32, name="b1r")
            nc.sync.dma_start(b1r[:], b1r_d[:])
            b2r = res.tile([P, OUT], F32, name="b2r")
            nc.sync.dma_start(b2r[:], b2r_d[:])
            ident = res.tile([P, P], F32, name="ident")
            make_identity(nc, ident[:])
            # PE warmup: dense dummy matmuls depending only on ident, so they
            # run during the initial DMA load and ramp the PE clock to 2.4GHz
            # before hop 1's real matmuls arrive.
            idb = res.tile([P, P], BF16, name="idb")
            nc.vector.tensor_copy(idb[:], ident[:])
            wps = pterm.tile([P, P], F32, tag="tp", bufs=2, name="warm_ps")
            for w in range(120):
                nc.tensor.matmul(wps[:], idb[:], idb[:], start=(w == 0),
                                 stop=(w == 119))

            # persistent per-block tensors
            h_t = [res.tile([P, F], F32, tag=f"h{m}", name=f"h{m}") for m in range(MB)]
            tx1T = [None] * MB   # [2 x (128,128) bf16] per block
            hw_all = [None] * MB

            # AG bounce buffers: [round][half]
            HR = MBH * P  # rows per half-shard = 640
            AGW = [F, F, OUT]  # payload width per AG round
            ag_in = [[dram.tile([HR, AGW[i]], BF16, name=f"agin{i}{h}")
                      for h in range(2)] for i in range(3)]
            ag_out = [[dram.tile([CORES * HR, AGW[i]], BF16, name=f"agout{i}{h}")
                       for h in range(2)] for i in range(3)]

            def stage_ag(i, mb, src_ap):
                half, m = mb // MBH, mb % MBH
                nc.sync.dma_start(ag_in[i][half][m * P:(m + 1) * P, :], src_ap)

            def emit_ag(i, half):
                nc.gpsimd.collective_compute(
                    "AllGather", mybir.AluOpType.bypass,
                    replica_groups=[list(range(CORES))],
                    ins=[ag_in[i][half][:].opt()],
                    outs=[ag_out[i][half][:].opt()],
                )

            def reload_u(i):
                # u[kt] rows are core (kt*128)//1280, m-block kt%10
                n_cols = AGW[i]
                for kt in range(KT):
                    c8, m = kt // MB, kt % MB
                    half, mh = (0, m) if m < MBH else (1, m - MBH)
                    src = ag_out[i][half][c8 * HR + mh * P: c8 * HR + (mh + 1) * P, :]
                    nc.sync.dma_start(u_tiles[kt][:, :n_cols], src)

            # ============ LAYER 1 ============
            # hop 1: Tx1 = -dis * (AT^T @ u0); stage dis*Tx1; keep Tx1^T bf16
            for mb in range(MB):
                pp = _emit_prop(nc, pprop, at_tiles, u_tiles, mb, F, split=False)
                tx1 = wrk.tile([P, F], F32, tag="tx1", bufs=2, name=f"tx1_{mb}")
                nc.vector.tensor_scalar_mul(tx1[:], pp[:], ndiso[:, mb:mb + 1])
                sc = wrk.tile([P, F], BF16, tag="sc", bufs=3, name=f"sc1_{mb}")
                nc.vector.tensor_scalar_mul(sc[:], tx1[:], diso[:, mb:mb + 1])
                stage_ag(0, mb, sc[:])
                tx1T[mb] = _transpose_to_bf16(nc, ptr, res, ident, tx1[:], mb, 2,
                                              "t1T", persistent=True)
                if mb == MBH - 1:
                    emit_ag(0, 0)
            emit_ag(0, 1)
            reload_u(0)

            # hop 2 + layer-1 combine -> h, z = [h@W2[1] | h@(2 W2[2])] staged for AG1
            for mb in range(MB):
                pp = _emit_prop(nc, pprop, at_tiles, u_tiles, mb, F, split=True)
                tmp = wrk.tile([P, F], F32, tag="tmp", bufs=2, name=f"tmp1_{mb}")
                nc.vector.tensor_scalar_mul(tmp[:], pp[:], n2diso[:, mb:mb + 1])
                xo = wrk.tile([P, F], F32, tag="xo", bufs=2, name=f"xo_{mb}")
                nc.sync.dma_start(xo[:], xo_r[mb])
                tx2 = wrk.tile([P, F], F32, tag="tx2", bufs=2, name=f"tx2_{mb}")
                nc.vector.tensor_sub(tx2[:], tmp[:], xo[:])
                tx2T = _transpose_to_bf16(nc, ptr, wrk, ident, tx2[:], mb, 2, "t2T")

                tp = pterm.tile([P, HID], F32, tag="tp", bufs=2, name=f"tp1_{mb}")
                sl = slice(mb * P, (mb + 1) * P)
                nc.tensor.matmul(tp[:], xoT_t[0][:, sl], w1t[0][0][:], start=True, stop=False)
                nc.tensor.matmul(tp[:], xoT_t[1][:, sl], w1t[0][1][:], start=False, stop=False)
                nc.tensor.matmul(tp[:], tx1T[mb][0][:], w1t[1][0][:], start=False, stop=False)
                nc.tensor.matmul(tp[:], tx1T[mb][1][:], w1t[1][1][:], start=False, stop=False)
                nc.tensor.matmul(tp[:], tx2T[0][:], w1t[2][0][:], start=False, stop=False)
                nc.tensor.matmul(tp[:], tx2T[1][:], w1t[2][1][:], start=False, stop=True)

                h = h_t[mb]
                nc.vector.tensor_add(h[:], tp[:], b1r[:])
                nc.vector.tensor_scalar_max(h[:], h[:], 0.0)
                # z = [h@W2[1] | h@(2*W2[2])]  (layer-2 pre-propagation features)
                hT = _transpose_to_bf16(nc, ptr, wrk, ident, h[:], mb, 2, "hT")
                zp = pterm.tile([P, F], F32, tag="tp", bufs=2, name=f"zp_{mb}")
                nc.tensor.matmul(zp[:, 0:OUT], hT[0][:], w2t[1][0][:], start=True, stop=False)
                nc.tensor.matmul(zp[:, 0:OUT], hT[1][:], w2t[1][1][:], start=False, stop=True)
                nc.tensor.matmul(zp[:, OUT:F], hT[0][:], w2t[2][0][:], start=True, stop=False)
                nc.tensor.matmul(zp[:, OUT:F], hT[1][:], w2t[2][1][:], start=False, stop=True)
                # hw = h @ (W2[0]-W2[2]) -> keep for final combine
                hw = res.tile([P, OUT], F32, tag=f"hw{mb}", name=f"hw_{mb}")
                hw_all[mb] = hw
                hwp = ptr.tile([P, OUT], F32, tag="trps", bufs=2, name=f"hwp_{mb}")
                nc.tensor.matmul(hwp[:], hT[0][:], w2t[0][0][:], start=True, stop=False)
                nc.tensor.matmul(hwp[:], hT[1][:], w2t[0][1][:], start=False, stop=True)
                nc.vector.tensor_copy(hw[:], hwp[:])
                sc = wrk.tile([P, F], BF16, tag="sc", bufs=3, name=f"sc2_{mb}")
                nc.vector.tensor_scalar_mul(sc[:], zp[:], diso[:, mb:mb + 1])
                stage_ag(1, mb, sc[:])
                if mb == MBH - 1:
                    emit_ag(1, 0)
            emit_ag(1, 1)
            reload_u(1)

            # ============ LAYER 2 ============
            # hop 3: Lz = -dis*(AT^T @ u); col 0:128 = Lz1 (final term),
            #        col 128:256 = Lz2 -> scale+stage for AG2
            lz1 = [res.tile([P, OUT], F32, tag=f"lz1_{m}", name=f"lz1_{m}")
                   for m in range(MB)]
            for mb in range(MB):
                pp = _emit_prop(nc, pprop, at_tiles, u_tiles, mb, F, split=True)
                lz = wrk.tile([P, F], F32, tag="lz", bufs=2, name=f"lz_{mb}")
                nc.vector.tensor_scalar_mul(lz[:], pp[:], ndiso[:, mb:mb + 1])
                nc.vector.tensor_copy(lz1[mb][:], lz[:, 0:OUT])
                sc = wrk.tile([P, OUT], BF16, tag="sc3", bufs=3, name=f"sc3_{mb}")
                nc.vector.tensor_scalar_mul(sc[:], lz[:, OUT:F], diso[:, mb:mb + 1])
                stage_ag(2, mb, sc[:])
                if mb == MBH - 1:
                    emit_ag(2, 0)
            emit_ag(2, 1)
            reload_u(2)

            # hop 4 (128 cols) + final combine
            for mb in range(MB):
                pp = _emit_prop(nc, pprop, at_tiles, u_tiles, mb, OUT, split=True)
                oacc = wrk.tile([P, OUT], F32, tag="oacc", bufs=3, name=f"oacc_{mb}")
                # oacc = -dis*pp + lz1 + hw + b2
                nc.vector.tensor_scalar_mul(oacc[:], pp[:], ndiso[:, mb:mb + 1])
                nc.vector.tensor_add(oacc[:], oacc[:], lz1[mb][:])
                nc.vector.tensor_add(oacc[:], oacc[:], hw_all[mb][:])
                nc.vector.tensor_add(oacc[:], oacc[:], b2r[:])
                nc.sync.dma_start(out_d[mb * P:(mb + 1) * P, :], oacc[:])

    nc.compile()
    return nc


def _prepare_inputs(x, edge, W1, b1, W2, b2):
    x = np.asarray(x, np.float32)
    edge = np.asarray(edge)
    W1 = np.asarray(W1, np.float32)
    b1 = np.asarray(b1, np.float32)
    W2 = np.asarray(W2, np.float32)
    b2 = np.asarray(b2, np.float32)
    src = edge[0].astype(np.int64)
    dst = edge[1].astype(np.int64)

    deg = np.bincount(dst, minlength=N).astype(np.float32)
    dis = np.where(deg > 0, 1.0 / np.sqrt(np.maximum(deg, 1.0)), 0.0).astype(np.float32)

    # dense transposed adjacency counts AT[s, d]
    flat = src * NP + dst
    uniq, cnt = np.unique(flat, return_counts=True)
    at8 = np.zeros(NP * NP, dtype=ml_dtypes.float8_e4m3)
    at8[uniq] = cnt.astype(ml_dtypes.float8_e4m3)
    at8 = at8.reshape(NP, NP)

    dis_pad = np.zeros(NP, np.float32)
    dis_pad[:N] = dis
    u0 = np.zeros((NP, F), ml_dtypes.bfloat16)
    u0[:N] = (dis[:, None] * x).astype(ml_dtypes.bfloat16)

    x_pad = np.zeros((NP, F), np.float32)
    x_pad[:N] = x

    w1b = W1.astype(ml_dtypes.bfloat16)
    w2x = np.stack([W2[0] - W2[2], W2[1], 2.0 * W2[2]]).astype(ml_dtypes.bfloat16)
    b1r = np.broadcast_to(b1, (P, HID)).copy()
    b2r = np.broadcast_to(b2, (P, OUT)).copy()

    in_maps = []
    for c in range(CORES):
        rows = slice(c * RPC, (c + 1) * RPC)
        dv = dis_pad[rows]
        m = {
            "at": np.ascontiguousarray(at8[:, rows]),
            "u0": u0,
            "xo": np.ascontiguousarray(x_pad[rows]),
            "xoT": np.ascontiguousarray(x_pad[rows].T).astype(ml_dtypes.bfloat16),
            "diso": np.ascontiguousarray(dv.reshape(MB, P).T),
            "ndiso": np.ascontiguousarray((-dv).reshape(MB, P).T),
            "n2diso": np.ascontiguousarray((-2.0 * dv).reshape(MB, P).T),
            "w1": w1b,
            "w2x": w2x,
            "b1r": b1r,
            "b2r": b2r,
        }
        in_maps.append(m)
    return in_maps


def _run(in_maps, trace=False, **kw):
    if "nc" not in _STATE:
        _STATE["nc"] = _build()
    r = run_bass_kernel_spmd(_STATE["nc"], in_maps, core_ids=list(range(CORES)),
                             trace=trace, **kw)
    out = np.concatenate([r.results[c]["outo"] for c in range(CORES)], axis=0)
    return out[:N], r


def kernel(**inputs) -> np.ndarray:
    in_maps = _prepare_inputs(**inputs)
    out, _ = _run(in_maps)
    return out
